# revision 86
# baseline (speedup 1.0000x reference)
# Multi-head attention (B=4, S=2048, D=1024, H=16) on 8 NeuronCores.
#
# Sharding: batch x head-group. Core c handles batch b=c//2 and heads
# 8*(c%2) .. 8*(c%2)+7 (a 512-wide slice of the model dim). Each core
# computes QKV projections for its slice, causal attention for its 8
# heads, and a row-parallel partial of the output projection. The host
# sums the two partials per batch and adds bo.
#
# Precision/engine split:
#  - Q/K projections and the QK^T scores run in fp8 e4m3 with DoubleRow
#    perf mode (two contraction tiles per pass). x and Wq/Wk are scaled
#    by 16 on the host so their products sit in e4m3's happy range; the
#    combined 256x score scale is folded into the softmax exp scale.
#    Softmax is insensitive to the ~2-3% fp8 score noise (it perturbs
#    probabilities multiplicatively and averages out over the value
#    contraction).
#  - The value path (x/Wv/V/probs/context/Wo all bf16) keeps higher
#    precision: value-path error lands directly in the output.
#  - k's bias is dropped: it shifts every score for a given query
#    equally, which softmax removes. q's bias is folded into the
#    PSUM->SBUF fp8 cast (one tensor_scalar mult+add on DVE).
#  - The q/k fp8 SBUF layout packs each head's 64 contraction dims as
#    [32 partitions x 2 DoubleRow k-tiles], four heads per 128
#    partitions, via a host-side permutation of the Wq/Wk columns (the
#    same permutation on q and k leaves q.k unchanged), so scores need
#    no on-chip reshuffling.
#  - Softmax denominators come from 64 ones-columns carried in each
#    head's V block (the AV matmul accumulates them on PSUM partitions
#    64..127); 1/sum runs on the DVE (InstReciprocal), keeping the
#    scalar engine exclusively on the big exp.
#  - Output partials are stored bf16 (the host upcasts and sums the two
#    per-batch partials); the quantization is ~0.002 absolute against a
#    0.028 gate and halves the store traffic.
#
# Causality is exploited at three granularities: fully-masked k-blocks
# are skipped, partially-masked diagonal blocks trim the scores/AV
# matmuls to the valid column range, and the in-block triangle is
# zeroed with one [128,128] upper-tri mask multiply.
#
# Emission is one continuous software-pipelined stream across all four
# q-chunks (no per-block drain): score groups stream per head with
# their AV matmuls lagging up to LAG=20 groups behind (24 et buffers),
# and projection/output-projection work is interleaved as PE filler.
# Explicit emission-order guards keep it correct: q/k cast producers
# are forced before any scores that read them, v-projection chunks
# before any AV touching their k-block, and output projection held
# until its q-chunk's context is complete. Input DMAs issue on
# the gpsimd queue in deadline order (x8_1 ahead of the value path --
# the scalar engine's q-chunk transitions hang off it); keeping the
# scalar sequencer free of DMA configs lets the tile scheduler place
# the exps early instead of hoisting DMA-gated filler ahead of them.
# The AV lag tapers to zero over the last q-chunk so the post-exp tail
# is just the final AV + output projection + store.
# Cost-model makespan ~187 us/core vs 204 us for the previous emission
# and 275 us for the fp32r baseline.

import sys

for _p in ("/opt/trn_rl_repo", "/root/.axon_site/_ro/trn_rl_repo"):
    if _p not in sys.path:
        sys.path.append(_p)

import ml_dtypes
import numpy as np

import concourse.bass as bass
import concourse.mybir as mybir
import concourse.tile as tile
from concourse.bass_utils import run_bass_kernel_spmd
from concourse.masks import make_upper_triangular

B, S, D, H = 4, 2048, 1024, 16
HD = D // H            # 64
N_CORES = 8
GH = 8                 # heads per core
C = GH * HD            # 512 local model dims per core
SCALE = HD ** -0.5
F32 = mybir.dt.float32
F32R = mybir.dt.float32r
BF16 = mybir.dt.bfloat16
FP8 = mybir.dt.float8e4
DR = mybir.MatmulPerfMode.DoubleRow
NP_FP8 = ml_dtypes.float8_e4m3

QK_SCALE = 16.0        # host-side scale on x and Wq/Wk before fp8 cast
import os as _os
WAIT_SCALE = float(_os.environ.get("KWAIT", "1.0"))

PE_NS = 1.0 / 2.4      # ns per matmul output row at full clock
ACT_NS = 1.0 / 1.2     # ns per activation element-row

T_CHUNK = 512          # t-tile for QKV projections
QC = 512               # q columns per attention chunk
KB = 128               # k rows per attention block
N_KB = S // KB         # 16
N_QC = S // QC         # 4
EXP_GROUP = int(_os.environ.get("KEG", "2"))  # k-blocks per batched exp


def _split_multi_waits(nc):
    """walrus in this container accepts only one sync-wait per instruction.
    Hoist all but the last wait of any multi-wait instruction onto NoOps
    inserted just before it on the same engine (sequencers execute their
    queue in order, so chained single waits are equivalent)."""
    for f in nc.m.functions:
        for blk in f.blocks:
            new_insts = []
            for inst in blk.instructions:
                si = inst.sync_info
                if si is not None and si.on_wait and len(si.on_wait) > 1:
                    waits = list(si.on_wait)
                    for i, w in enumerate(waits[:-1]):
                        nop = mybir.InstNoOp(name=f"{inst.name}_sw{i}", ins=[], outs=[])
                        nop.engine = inst.engine
                        nop.sync_info = mybir.SyncInfo(on_wait=[w], on_update=[])
                        new_insts.append(nop)
                    si.on_wait = [waits[-1]]
                new_insts.append(inst)
            blk.instructions[:] = new_insts


def _emit_kernel(nc, reps=1):
    xt = nc.dram_tensor("xt", [D, S], BF16, kind="ExternalInput").ap()
    x8t = nc.dram_tensor("x8t", [D, S], FP8, kind="ExternalInput").ap()
    wq8 = nc.dram_tensor("wq8", [D, C], FP8, kind="ExternalInput").ap()
    wk8 = nc.dram_tensor("wk8", [D, C], FP8, kind="ExternalInput").ap()
    wvt = nc.dram_tensor("wvt", [D, C], BF16, kind="ExternalInput").ap()
    bqv = nc.dram_tensor("bqv", [C], F32, kind="ExternalInput").ap()
    bvb = nc.dram_tensor("bvb", [128, C], F32, kind="ExternalInput").ap()
    wot = nc.dram_tensor("wot", [C, D], BF16, kind="ExternalInput").ap()
    out = nc.dram_tensor("out", [S, D], BF16, kind="ExternalOutput").ap()

    NFC = D // 128      # 8 f-chunks of the projection contraction
    NFP = NFC // 2      # 4 DoubleRow fc-pairs
    NCC = C // 128      # 4 c-chunks of the local model dim

    with tile.TileContext(nc) as tc:
        import contextlib

        ctx = contextlib.ExitStack()
        with ctx:
            consts = ctx.enter_context(tc.tile_pool(name="consts", bufs=1))
            wpool = ctx.enter_context(tc.tile_pool(name="wpool", bufs=1))
            qkv = ctx.enter_context(tc.tile_pool(name="qkv", bufs=1))
            xtp = ctx.enter_context(tc.tile_pool(name="xtp", bufs=2))
            expp = ctx.enter_context(tc.tile_pool(name="expp", bufs=int(_os.environ.get("KETB", "24"))))
            ctxp = ctx.enter_context(tc.tile_pool(name="ctxp", bufs=1))
            smallp = ctx.enter_context(tc.tile_pool(name="smallp", bufs=1))
            outp = ctx.enter_context(tc.tile_pool(name="outp", bufs=3))

            ps_qkv = ctx.enter_context(
                tc.tile_pool(name="ps_qkv", bufs=int(_os.environ.get("KPQKV", "2")), space="PSUM")
            )
            ps_sc = ctx.enter_context(
                tc.tile_pool(name="ps_sc", bufs=int(_os.environ.get("KPSC", "2")), space="PSUM")
            )
            ps_av = ctx.enter_context(
                tc.tile_pool(name="ps_av", bufs=int(_os.environ.get("KPAV", "2")), space="PSUM")
            )

            # ---- constants -------------------------------------------------
            tri = consts.tile([128, 128], BF16)      # tri[p, c] = 1.0 iff p <= c
            make_upper_triangular(nc, tri[:, :], val=1.0, diag=True)
            # two stacked copies for the merged diagonal-pair mask
            # multiply, built by DVE copies so the Pool queue (which
            # carries the input DMA configs) stays untouched
            tri2 = consts.tile([128, 2, 128], BF16)
            nc.vector.tensor_copy(tri2[:, 0, :], tri[:, :])
            nc.vector.tensor_copy(tri2[:, 1, :], tri[:, :])

            # p-state primer: the tensor engine's clock ramps over ~3us of
            # continuous execution. The PE would otherwise sit idle until
            # the first input DMAs land (~7us) and then crawl through the
            # warmup at the low p-state; a burst of dummy matmuls on the
            # tri constant (no DMA dependency) keeps the PE busy from t=0
            # so the real projections start fully ramped.
            ps_warm = ps_qkv.tile([128, T_CHUNK], F32, name="ps_qk", tag="ps_qkv")
            for _ in range(int(_os.environ.get("KPRIME", "24"))):
                nc.tensor.matmul(ps_warm[:, 0:128], tri[:, :], tri[:, :],
                                 start=True, stop=True)

            # ---- inputs: one queue (gpsimd), strict deadline order --------
            # Transfers serialize within a queue, so ordering is controlled
            # by putting all inputs on one queue. gpsimd (not scalar): a
            # long config stream on the scalar sequencer makes the tile
            # scheduler model the exps as late and hoist DMA-gated filler
            # ahead of them. First-exp path (x8 head, wq8/wk8 first halves,
            # bq), then x8_1 (next q-chunk's scores) ahead of the value
            # path (xt0, wv), then the rest by next-use time.
            x8_sb = wpool.tile([128, NFC, S], FP8)   # full 16*x, fp8
            _x8_ap = x8t.rearrange("(fc p) t -> p fc t", p=128)
            nc.gpsimd.dma_start(out=x8_sb[:, :, 0 : S // 4], in_=_x8_ap[:, :, 0 : S // 4])
            wq8_sb = wpool.tile([128, NFC, C], FP8)
            wk8_sb = wpool.tile([128, NFC, C], FP8)
            _wq8_ap = wq8.rearrange("(fc p) c -> p fc c", p=128)
            _wk8_ap = wk8.rearrange("(fc p) c -> p fc c", p=128)
            # wq8/wk8 split in C-halves so the first scores' DMA chain is
            # minimal: warmup only needs cols 0..255 (chunk-pair e=0).
            nc.gpsimd.dma_start(out=wq8_sb[:, :, 0 : C // 2], in_=_wq8_ap[:, :, 0 : C // 2])
            nc.gpsimd.dma_start(out=wk8_sb[:, :, 0 : C // 2], in_=_wk8_ap[:, :, 0 : C // 2])
            bq_sb = consts.tile([128, NCC], F32)    # 16*bq[perm][cc*128 + p] at [p, cc]
            nc.gpsimd.dma_start(out=bq_sb[:, :], in_=bqv.rearrange("(cc p) -> p cc", p=128))

            _xt_pref = {}

            def _xt_prefetch(tci, queue):
                xt_c = xtp.tile([128, NFC, T_CHUNK], BF16, name="xt_c")
                queue.dma_start(
                    out=xt_c[:, :, :],
                    in_=xt.rearrange("(fc p) t -> p fc t", p=128)[
                        :, :, tci * T_CHUNK : (tci + 1) * T_CHUNK
                    ],
                )
                _xt_pref[tci] = xt_c

            nc.gpsimd.dma_start(out=wq8_sb[:, :, C // 2 : C], in_=_wq8_ap[:, :, C // 2 : C])
            nc.gpsimd.dma_start(out=wk8_sb[:, :, C // 2 : C], in_=_wk8_ap[:, :, C // 2 : C])
            nc.gpsimd.dma_start(out=x8_sb[:, :, S // 4 : S // 2], in_=_x8_ap[:, :, S // 4 : S // 2])
            _xt_prefetch(0, nc.gpsimd)
            wv_sb = wpool.tile([128, NFC, C], BF16)
            nc.gpsimd.dma_start(out=wv_sb[:, :, :], in_=wvt.rearrange("(fc p) c -> p fc c", p=128))
            bv_bc = consts.tile([128, C], F32)      # bv broadcast across partitions
            nc.gpsimd.dma_start(out=bv_bc[:, :], in_=bvb)
            _xt_prefetch(1, nc.gpsimd)
            nc.gpsimd.dma_start(out=x8_sb[:, :, S // 2 : 3 * S // 4], in_=_x8_ap[:, :, S // 2 : 3 * S // 4])
            nc.gpsimd.dma_start(out=x8_sb[:, :, 3 * S // 4 : S], in_=_x8_ap[:, :, 3 * S // 4 : S])
            wo_sb = wpool.tile([128, NCC, D], BF16)
            nc.gpsimd.dma_start(out=wo_sb[:, :, :], in_=wot.rearrange("(cc p) d -> p cc d", p=128))

            # ---- persistent activations -----------------------------------
            # q/k fp8: [32*quad partitions, chunk-pair e, d-half j, t];
            # head h = 4e + quad, its 64 d-dims at partitions 32*quad..+32,
            # split into two DoubleRow k-tiles along j.
            qt8_sb = qkv.tile([128, 2, 2, S], FP8)
            kt8_sb = qkv.tile([128, 2, 2, S], FP8)
            # APs cannot start at partition 96, so quad-3 heads (h=3,7) get
            # their q/k relocated to partitions 0-63 of side tiles via
            # SBUF->SBUF DMA (the only engine that can shift partitions).
            qx_sb = qkv.tile([64, 2, S], FP8)   # [32*e + r, d-half, t]
            kx_sb = qkv.tile([64, 2, S], FP8)
            v_sb = qkv.tile([128, N_KB, GH, 2 * HD], BF16)  # v + 64 ones cols
            ctx_sb = ctxp.tile([128, NCC, S], BF16)  # ctxT: [c within chunk, cc, q]

            nc.gpsimd.memset(v_sb[:, :, :, HD : 2 * HD], 1.0)

            _TRI_ENG = nc.gpsimd if _os.environ.get("KTRI", "0") == "1" else nc.vector
            _VADD_ENG = nc.gpsimd if _os.environ.get("KVADD", "0") == "1" else nc.vector

            _phases = "123"
            _xt_tiles = {}

            def emit_xt_load(tci):
                if tci not in _xt_pref:
                    _xt_prefetch(tci, nc.sync)
                _xt_tiles[tci] = _xt_pref.pop(tci)
                return 0.0

            def emit_qk_group(tci, is_q, cc, cast_on_act=False):
                """One 128-col psum group of the q (or k) fp8 DoubleRow
                projection + its PSUM->fp8 SBUF cast; quad-3 relocation DMA
                after the last chunk."""
                t0 = tci * T_CHUNK
                w_sb, y_sb = (wq8_sb, qt8_sb) if is_q else (wk8_sb, kt8_sb)
                e, j = cc // 2, cc % 2
                ps = ps_qkv.tile([128, T_CHUNK], F32, name="ps_qk", tag="ps_qkv")
                # moving free dim caps at 512 (2x256 under DoubleRow), so a
                # 512-t chunk takes two matmuls per fc-pair
                TH = 256
                for th in range(T_CHUNK // TH):
                    ts0 = t0 + th * TH
                    for fp in range(NFP):
                        nc.tensor.matmul(
                            ps[:, th * TH : (th + 1) * TH],
                            w_sb[:, 2 * fp : 2 * fp + 2, cc * 128 : (cc + 1) * 128],
                            x8_sb[:, 2 * fp : 2 * fp + 2, ts0 : ts0 + TH],
                            start=(fp == 0),
                            stop=(fp == NFP - 1),
                            perf_mode=DR,
                        )
                # psum holds 256*(proj); store 16*(proj+bias) as fp8
                if is_q:
                    nc.vector.tensor_scalar(
                        y_sb[:, e, j, t0 : t0 + T_CHUNK],
                        ps[:, :],
                        1.0 / QK_SCALE,
                        bq_sb[:, cc : cc + 1],
                        op0=mybir.AluOpType.mult,
                        op1=mybir.AluOpType.add,
                    )
                elif cast_on_act:
                    # warmup-only: the scalar engine is idle before the
                    # first exp, and the DVE's wake latency sits on the
                    # critical path to the first scores
                    nc.scalar.activation(
                        y_sb[:, e, j, t0 : t0 + T_CHUNK],
                        ps[:, :],
                        mybir.ActivationFunctionType.Copy,
                        bias=0.0,
                        scale=1.0 / QK_SCALE,
                    )
                else:
                    nc.vector.tensor_scalar_mul(
                        y_sb[:, e, j, t0 : t0 + T_CHUNK], ps[:, :], 1.0 / QK_SCALE
                    )
                if j == 1:
                    # both d-halves of chunk-pair e are now cast: relocate
                    # its quad-3 head rows (base partition 96 is not
                    # AP-addressable) to the side tile
                    x_sb = qx_sb if is_q else kx_sb
                    nc.sync.dma_start(
                        out=x_sb[32 * e : 32 * e + 32, :, t0 : t0 + T_CHUNK],
                        in_=y_sb[96:128, e, :, t0 : t0 + T_CHUNK],
                    )
                return NFP * T_CHUNK * 0.5 * PE_NS

            def emit_v_group(tci, tt, half):
                """Half-width (4-head) v projection for one 128-t slice:
                853 ns of PE — small enough to fit scalar-engine slack
                windows, so v work doesn't pile up at q-chunk boundaries."""
                t0 = tci * T_CHUNK
                if tci not in _xt_tiles:
                    emit_xt_load(tci)
                xt_c = _xt_tiles[tci]
                kb = (t0 + tt * 128) // KB
                c0 = half * (C // 2)
                h0 = half * (GH // 2)
                ps = ps_qkv.tile([128, C // 2], F32, name="ps_v", tag="ps_qkv")
                for fc in range(NFC):
                    nc.tensor.matmul(
                        ps[:, :],
                        xt_c[:, fc, tt * 128 : (tt + 1) * 128],
                        wv_sb[:, fc, c0 : c0 + C // 2],
                        start=(fc == 0),
                        stop=(fc == NFC - 1),
                    )
                _VADD_ENG.tensor_add(
                    v_sb[:, kb, h0 : h0 + GH // 2, 0:HD],
                    ps.rearrange("p (h d) -> p h d", h=GH // 2),
                    bv_bc.rearrange("p (h d) -> p h d", h=GH)[:, h0 : h0 + GH // 2, :],
                )
                if tt == T_CHUNK // 128 - 1 and half == 1:
                    _xt_tiles.pop(tci)
                return NFC * (C // 2) * PE_NS

            av_tiles = {}

            def eg_of(qi):
                return EXP_GROUP

            def sc_widths(qi, gi):
                """Matmul column-chunk widths for scores group gi at qi."""
                eg = eg_of(qi)
                nkb = 4 * qi + 4
                g_min = gi * eg - 4 * qi
                g_off0 = 128 * g_min if g_min > 0 else 0
                widths = []
                for kb in range(gi * eg, min((gi + 1) * eg, nkb)):
                    qq = g_off0
                    while qq < QC:
                        w = min(256, QC - qq)
                        widths.append(w)
                        qq += w
                return widths

            def emit_sc_group(h, qi, gi):
                """Scores + exp + tri-mask for k-group gi of head h. Returns
                the state emit_av_group needs, so AV can lag one group behind
                and the PE never stalls on the scalar engine's exp."""
                e, a = h // 4, h % 4
                if a < 3:
                    p0 = 32 * a
                    q_ap = lambda c0, c1: qt8_sb[p0 : p0 + 32, e, :, c0:c1]
                    k_ap = lambda c0, c1: kt8_sb[p0 : p0 + 32, e, :, c0:c1]
                else:
                    p0 = 32 * e
                    q_ap = lambda c0, c1: qx_sb[p0 : p0 + 32, :, c0:c1]
                    k_ap = lambda c0, c1: kx_sb[p0 : p0 + 32, :, c0:c1]
                q0 = qi * QC
                eg = eg_of(qi)
                nkb = 4 * qi + 4
                kb_lo = gi * eg
                kb_hi = min(kb_lo + eg, nkb)
                gw = kb_hi - kb_lo
                if gi == 0:
                    av_tiles[h] = ps_av.tile([128, QC], F32, name="av_ps")
                sc_ps = ps_sc.tile([128, eg, QC], F32)
                g_min = kb_lo - 4 * qi
                g_off0 = 128 * g_min if g_min > 0 else 0
                for kb in range(kb_lo, kb_hi):
                    # write from the group's min offset so the grouped exp
                    # below never reads uninitialized psum; from qi 1 on,
                    # every psum column was written by an earlier group, so
                    # the diagonal blocks can trim to their own offset (the
                    # exp of the stale strip is finite and never read by AV)
                    m = kb - 4 * qi
                    off = max(g_off0, 128 * m) if (qi > 0 and m > 0) else g_off0
                    qq = off
                    while qq < QC:
                        w = min(256, QC - qq)
                        nc.tensor.matmul(
                            sc_ps[:, kb - kb_lo, qq : qq + w],
                            k_ap(kb * KB, (kb + 1) * KB),
                            q_ap(q0 + qq, q0 + qq + w),
                            start=True,
                            stop=True,
                            perf_mode=DR,
                        )
                        qq += w
                et = expp.tile([128, eg, QC], BF16)
                # cols < 128*m of diagonal block m are never read by
                # AV; a rectangular trim to the group's min offset is
                # safe and cuts ACT work on the causal tail.
                g_min_m = kb_lo - 4 * qi
                g_off = 128 * g_min_m if g_min_m > 0 else 0
                nc.scalar.activation(
                    et[:, 0:gw, g_off:QC],
                    sc_ps[:, 0:gw, g_off:QC],
                    mybir.ActivationFunctionType.Exp,
                    bias=0.0,
                    scale=SCALE / (QK_SCALE * QK_SCALE),
                )
                m0 = kb_lo - 4 * qi
                if gw == 2 and m0 >= 0 and _os.environ.get("KTRI2", "1") == "1":
                    # both k-blocks diagonal: one mask multiply over a
                    # diagonal-stride AP (slot stride QC+128 walks the slot
                    # AND the 128-col offset together), halving the DVE
                    # instruction count on the causal diagonal
                    import dataclasses as _dc
                    base = et[:, 0:2, 128 * m0 : 128 * m0 + 128]
                    _ap = [list(p) for p in base.ap]
                    _ap[1][0] += 128
                    diag = _dc.replace(base, ap=type(base.ap)(_ap))
                    _TRI_ENG.tensor_mul(diag, diag, tri2[:, :, :])
                else:
                    for kb in range(kb_lo, kb_hi):
                        m = kb - 4 * qi
                        if m >= 0:
                            off = 128 * m
                            _TRI_ENG.tensor_mul(
                                et[:, kb - kb_lo, off : off + 128],
                                et[:, kb - kb_lo, off : off + 128],
                                tri[:, :],
                            )
                exp_ns = gw * (QC - g_off) * ACT_NS + 228.0
                return (h, qi, gi, et, kb_lo, kb_hi, nkb), exp_ns

            def emit_av_group(state):
                h, qi, gi, et, kb_lo, kb_hi, nkb = state
                av_ps = av_tiles[h]
                pe_ns = 0.0
                for kb in range(kb_lo, kb_hi):
                    m = kb - 4 * qi
                    off = 128 * m if m >= 0 else 0
                    nc.tensor.matmul(
                        av_ps[:, off:QC],
                        v_sb[:, kb, h, :],
                        et[:, kb - kb_lo, off:QC],
                        start=(kb == 0),
                        stop=(kb == nkb - 1),
                    )
                    pe_ns += (QC - off) * PE_NS
                if kb_hi == nkb:
                    q0 = qi * QC
                    rbc = smallp.tile([HD, QC], F32)
                    hc = h // 2         # ctx keeps natural head order
                    hp = 64 * (h % 2)
                    if qi == N_QC - 1 and h == HEAD_ORDER[-1]:
                        # very last head: 128-col pieces so the first output
                        # projection chunks start as soon as their q-slice
                        # of ctx lands instead of after the full 512
                        for qp in range(0, QC, 128):
                            nc.vector.reciprocal(
                                rbc[:, qp : qp + 128],
                                av_ps[HD : 2 * HD, qp : qp + 128],
                            )
                            nc.vector.tensor_mul(
                                ctx_sb[hp : hp + HD, hc, q0 + qp : q0 + qp + 128],
                                av_ps[0:HD, qp : qp + 128],
                                rbc[:, qp : qp + 128],
                            )
                    else:
                        with tc.high_priority(-int(_os.environ.get("KRCPD", "0"))):
                            nc.vector.reciprocal(rbc[:, :], av_ps[HD : 2 * HD, :])
                            nc.vector.tensor_mul(
                                ctx_sb[hp : hp + HD, hc, q0 : q0 + QC],
                                av_ps[0:HD, :],
                                rbc[:, :],
                            )
                    del av_tiles[h]
                return pe_ns

            _o_tiles = {}

            def emit_ph3_group(qq, eh, split_dma=False):
                ps = ps_qkv.tile([128, D // 2], F32, name="ps_op", tag="ps_qkv")
                for cc in range(NCC):
                    nc.tensor.matmul(
                        ps[:, :],
                        ctx_sb[:, cc, qq * 128 : (qq + 1) * 128],
                        wo_sb[:, cc, eh * (D // 2) : (eh + 1) * (D // 2)],
                        start=(cc == 0),
                        stop=(cc == NCC - 1),
                    )
                if eh == 0:
                    _o_tiles[qq] = outp.tile([128, D], BF16, name="o_sb")
                o_sb = _o_tiles[qq]
                _OCP = nc.gpsimd if _os.environ.get("KOCP", "0") == "1" else nc.vector
                _OCP.tensor_copy(o_sb[:, eh * (D // 2) : (eh + 1) * (D // 2)], ps[:, :])
                if split_dma:
                    # final chunk: store each half right after its copy so
                    # the last DMA is half-size and starts a copy earlier
                    nc.sync.dma_start(
                        out=out[qq * 128 : (qq + 1) * 128,
                                eh * (D // 2) : (eh + 1) * (D // 2)],
                        in_=o_sb[:, eh * (D // 2) : (eh + 1) * (D // 2)],
                    )
                    if eh == 1:
                        _o_tiles.pop(qq)
                elif eh == 1:
                    nc.sync.dma_start(
                        out=out[qq * 128 : (qq + 1) * 128, :],
                        in_=_o_tiles.pop(qq)[:, :],
                    )
                return NCC * (D // 2) * PE_NS

            TPQ = QC // T_CHUNK  # t-chunks per attention q-chunk
            QQP = QC // 128      # out-proj 128-row chunks per q-chunk
            # quad-3 heads last: their scores wait on the relocation DMA
            HEAD_ORDER = [0, 1, 2, 4, 5, 6, 3, 7]

            # Estimated arrival times of the serialized scalar-queue input
            # DMAs (all engines' queues share one HWDGE + DMA pipe, so
            # emission order == transfer order). Used to gate filler: work
            # emitted before its inputs exist head-of-line-blocks an
            # in-order queue.
            def _dma_ns(nbytes, elem):
                mult = 2.0 if elem < 512 else 1.0
                return (nbytes / elem) / 16.0 * max(elem * mult / 22.5, 7.0)

            _arr = {}
            _dma_clk = 2300.0
            for _nm, _nb, _el in (
                ("x8_0", D * S // 4, 512), ("wq8a", D * C // 2, 256),
                ("wk8a", D * C // 2, 256), ("bq", C * 4, 2048),
                ("wq8b", D * C // 2, 256), ("wk8b", D * C // 2, 256),
                ("x8_1", D * S // 4, 512), ("xt0", D * T_CHUNK * 2, 1024),
                ("wv", D * C * 2, 1024), ("bv", 128 * C * 4, 2048),
                ("xt1", D * T_CHUNK * 2, 1024), ("x8_2", D * S // 4, 512),
                ("x8_3", D * S // 4, 512), ("wo", C * D * 2, 2048),
            ):
                _dma_clk += _dma_ns(_nb, _el) + 90.0
                _arr[_nm] = _dma_clk
            _xt_arr = {0: _arr["xt0"], 1: _arr["xt1"]}

            def _ready_qk(tci, is_q, cc):
                xq = (tci * T_CHUNK) // (S // 4)
                w = ("wq8" if is_q else "wk8") + ("a" if cc < 2 else "b")
                return max(_arr[f"x8_{xq}"], _arr[w])

            clk = {"pe": 0.0, "act": 0.0}

            def run(ch):
                if ch["done"]:
                    return
                ch["done"] = True
                clk["pe"] += ch["fn"]()

            def force(ch):
                if not ch["done"]:
                    clk["pe"] = max(clk["pe"], ch["ready"]())
                    run(ch)

            for _rep in range(reps):
                # warmup: only the e=0 q/k chunks of t-chunk 0 (all that
                # heads 0-2 of block 0 need); the e=1 chunks run as
                # demand-forced filler before head 4's items.
                emit_xt_load(0)
                clk["pe"] = max(clk["pe"], _arr["wq8a"])
                if _os.environ.get("KWU", "0") == "1":
                    # interleave q/k so each side's cast drains while the
                    # other side's matmuls occupy its psum buffer
                    clk["pe"] += emit_qk_group(0, True, 0)
                    clk["pe"] = max(clk["pe"], _arr["wk8a"])
                    clk["pe"] += emit_qk_group(0, False, 0)
                    clk["pe"] += emit_qk_group(0, True, 1)
                    clk["pe"] += emit_qk_group(0, False, 1)
                else:
                    for cc in range(2):
                        clk["pe"] += emit_qk_group(0, True, cc)
                    clk["pe"] = max(clk["pe"], _arr["wk8a"])
                    for cc in range(2):
                        clk["pe"] += emit_qk_group(
                            0, False, cc,
                            cast_on_act=_os.environ.get("KACT", "0") == "1",
                        )

                filler = []   # opportunistic PE work, popped in list order
                forced = []   # v chunks, demanded by AV k-block coverage
                pend = []     # score groups awaiting their AV emission
                qk_pend = {}  # (tci, is_q, cc) -> chunk, for scores guards
                xt_pend = {}  # tci -> chunk
                n_ctx = {}    # qi -> heads with final ctx written
                ctx_done = {}  # qi -> est time all ctx available

                def add_xt(tci):
                    def fn():
                        xt_pend.pop(tci, None)
                        emit_xt_load(tci)
                        _xt_arr[tci] = clk["pe"] + 4400.0
                        return 0.0
                    ch = {"ready": lambda: 0.0, "fn": fn, "cost": 0.0,
                          "done": False}
                    xt_pend[tci] = ch
                    filler.append(ch)

                def add_qk(tci, is_q, cc):
                    def fn(t=tci, q=is_q, c=cc):
                        qk_pend.pop((t, q, c), None)
                        with tc.tile_wait_until(_ready_qk(t, q, c) * 1e-6 * WAIT_SCALE):
                            return emit_qk_group(t, q, c)
                    _fl = (float(_os.environ.get("KG23", "0"))
                           if (tci == 0 and cc >= 2) else 0.0)
                    ch = {"ready": (lambda t=tci, q=is_q, c=cc, f=_fl:
                                    max(f, _ready_qk(t, q, c))),
                          "fn": fn, "cost": NFP * T_CHUNK * 0.5 * PE_NS,
                          "done": False}
                    qk_pend[(tci, is_q, cc)] = ch
                    filler.append(ch)

                _V_GATE = {3: float(_os.environ.get("KV3", "0"))}

                def add_v(tci, tt, half):
                    def rdy(t=tci):
                        return max(_xt_arr.get(t, clk["pe"] + 4400.0), _arr["wv"],
                                   _V_GATE.get(t, 0.0))

                    def fn(t=tci, s=tt, hf=half):
                        if t in xt_pend:
                            force(xt_pend[t])
                        with tc.tile_wait_until(rdy(t) * 1e-6 * WAIT_SCALE):
                            return emit_v_group(t, s, hf)
                    ch = {"ready": rdy, "fn": fn,
                          "cost": NFC * (C // 2) * PE_NS, "done": False,
                          "kb": tci * (T_CHUNK // 128) + tt}
                    forced.append(ch)
                    filler.append(ch)

                _PH3_GATE = {
                    0: float(_os.environ.get("KP0", "0")),
                    1: float(_os.environ.get("KP1", "0")),
                    2: float(_os.environ.get("KP2", "0")),
                }

                def add_ph3(qq, eh, src_qi):
                    def rdy(q=src_qi):
                        return max(_arr["wo"], ctx_done.get(q, 1e15),
                                   _PH3_GATE.get(q, 0.0))

                    def fn(q=qq, e=eh):
                        gate = max(_arr["wo"], _PH3_GATE.get(q, 0.0))
                        with tc.tile_wait_until(gate * 1e-6 * WAIT_SCALE):
                            return emit_ph3_group(q, e)
                    filler.append({"ready": rdy, "fn": fn,
                                   "cost": NCC * (D // 2) * PE_NS, "done": False})

                def pop_filler(limit=None):
                    """Emit the first filler chunk whose inputs have arrived;
                    returns False if none runnable (or over limit)."""
                    i = 0
                    while i < len(filler):
                        ch = filler[i]
                        if ch["done"]:
                            filler.pop(i)
                            continue
                        if ch["ready"]() <= clk["pe"] and not (
                            limit is not None
                            and clk["pe"] + ch["cost"] > limit + float(_os.environ.get("KLIM", "400"))
                        ):
                            run(ch)
                            filler.pop(i)
                            return True
                        i += 1
                    return False

                def flush_av():
                    state, act_end = pend.pop(0)
                    kb_hi = state[5]
                    while forced and (forced[0]["done"] or forced[0]["kb"] < kb_hi):
                        ch = forced.pop(0)
                        force(ch)
                    while clk["pe"] < act_end and pop_filler(limit=act_end):
                        pass
                    clk["pe"] = max(clk["pe"], act_end)
                    clk["pe"] += emit_av_group(state)
                    if state[5] == state[6]:      # head's last k-group: ctx out
                        sqi = state[1]
                        n_ctx[sqi] = n_ctx.get(sqi, 0) + 1
                        if n_ctx[sqi] == GH:
                            ctx_done[sqi] = clk["pe"] + 800.0

                def guard_qk(h, qi, gi):
                    """Scores read q/k casts of their t-ranges: force any
                    still-unemitted producer chunks (emission-order dep)."""
                    e = h // 4
                    eg = eg_of(qi)
                    for cc in (2 * e, 2 * e + 1):
                        ch = qk_pend.get((qi, True, cc))
                        if ch is not None:
                            force(ch)
                    nkb = 4 * qi + 4
                    for kb in range(gi * eg, min((gi + 1) * eg, nkb)):
                        tci = kb * KB // T_CHUNK
                        for cc in (2 * e, 2 * e + 1):
                            ch = qk_pend.get((tci, False, cc))
                            if ch is not None:
                                force(ch)

                # t-chunk 0 leftovers (e=1 projections + value path)
                for q in (True, False):
                    for c in (2, 3):
                        add_qk(0, q, c)
                for tt in range(T_CHUNK // 128):
                    for half in (0, 1):
                        add_v(0, tt, half)

                for qi in range(N_QC):
                    # stage the next q-chunk's projections + the previous
                    # q-chunk's output projection as filler; no boundary
                    # drain -- leftovers carry into the next q-chunk's flow.
                    if qi + 1 < N_QC:
                        for tci in range(TPQ * (qi + 1), TPQ * (qi + 2)):
                            if tci not in _xt_arr and tci not in _xt_pref:
                                add_xt(tci)
                            for q in (True, False):
                                for c in range(NCC):
                                    add_qk(tci, q, c)
                            for tt in range(T_CHUNK // 128):
                                for half in (0, 1):
                                    add_v(tci, tt, half)
                    if qi > 0:
                        for qq in range((qi - 1) * QQP, qi * QQP):
                            for eh in range(2):
                                add_ph3(qq, eh, qi - 1)
                    nkb = 4 * qi + 4
                    n_grp = (nkb + eg_of(qi) - 1) // eg_of(qi)
                    if _os.environ.get("KQKE", "0") == "1" and qi + 1 < N_QC:
                        # emit the next t-chunk's projections NOW regardless
                        # of DMA arrival: the scheduler orders by readiness
                        # anyway, and early emission gives their casts early
                        # DVE priority at the q-chunk transition
                        for q_ in (True, False):
                            for c_ in range(NCC):
                                ch = qk_pend.get((qi + 1, q_, c_))
                                if ch is not None and not ch["done"]:
                                    run(ch)
                    # qi 0: emit ALL score groups before any AV -- its AVs
                    # gate on the value-path DMAs, and the scalar engine
                    # must never sit behind a wv-stalled AV. The time-based
                    # flush is disabled there (DMA guards jump clk["pe"]
                    # past the first exps' end without the PE being busy).
                    LAG = 16 if qi == 0 else int(_os.environ.get("KLAG", "20"))
                    # taper: drain the AV backlog during the last Act-bound
                    # exps so the tail after the final exp is just the last
                    # AV + ph3 + the out DMA
                    n_left = GH * n_grp
                    idx0 = 0
                    for h in HEAD_ORDER:
                        for gi in range(n_grp):
                            # qk-cast producers first: the next exps hang off
                            # them, while the AV drain below is Act-independent
                            guard_qk(h, qi, gi)
                            if gi == 0:
                                # h's previous-q-chunk AV chain must drain
                                # before its av psum tile is re-allocated
                                while any(p[0][0] == h for p in pend):
                                    flush_av()
                            state, exp_ns = emit_sc_group(h, qi, gi)
                            clk["pe"] += sum(sc_widths(qi, gi)) * 0.5 * PE_NS
                            clk["act"] = max(clk["act"], clk["pe"] + 300.0) + exp_ns
                            pend.append((state, clk["act"]))
                            n_left -= 1
                            lag_eff = LAG
                            if qi == N_QC - 1:
                                lag_eff = min(LAG, max(0, n_left - 1))
                            while pend and (
                                len(pend) > lag_eff
                                or (qi > 0 and clk["pe"] >= pend[0][1])
                            ):
                                flush_av()
                            _tailp = int(_os.environ.get("KTAILP", "4"))
                            if (
                                (qi == 0 and idx0 >= int(_os.environ.get("KQ0P", "10")))
                                or (qi > 0 and _tailp > 0
                                    and idx0 >= GH * n_grp - _tailp)
                            ):
                                # late in q-chunk 0 the scalar engine is
                                # saturated and no AV flush windows exist;
                                # pop a ready filler chunk so the next
                                # t-chunk's qk casts get early DVE priority
                                pop_filler(limit=clk["pe"] + float(_os.environ.get("KPOPW", "100")))
                            idx0 += 1
                while pend:
                    flush_av()
                while True:
                    left = [c for c in filler if not c["done"]]
                    if not left:
                        break
                    if not pop_filler():
                        clk["pe"] = max(
                            clk["pe"] + 1.0, min(c["ready"]() for c in left)
                        )
                _last_qq = N_QC * QQP - 1
                for qq in range((N_QC - 1) * QQP, N_QC * QQP):
                    for eh in range(2):
                        emit_ph3_group(qq, eh, split_dma=(qq == _last_qq))

    _split_multi_waits(nc)
    return nc


_CACHED = {}


def _build(reps=1):
    if reps not in _CACHED:
        nc = bass.Bass("TRN2", target_bir_lowering=False, debug=False)
        _CACHED[reps] = _emit_kernel(nc, reps)
    return _CACHED[reps]


# q/k column permutation: local column cc*128 + 32*a + r holds head
# 4*(cc//2) + a, contraction dim 32*(cc%2) + r. Applying the same
# permutation to q and k leaves q.k (and so the scores) unchanged.
_QK_PERM = np.empty(C, np.int64)
for _cc in range(C // 128):
    for _a in range(4):
        for _r in range(32):
            _QK_PERM[_cc * 128 + 32 * _a + _r] = (
                (4 * (_cc // 2) + _a) * HD + 32 * (_cc % 2) + _r
            )


def _reference_numpy(x, Wq, bq, Wk, bk, Wv, bv, Wo, bo, attention_mask):
    """Fallback for non-all-ones attention masks (spec fills ones)."""
    scale = HD ** -0.5
    out = np.empty((B, S, D), np.float32)
    causal = np.triu(np.ones((S, S), bool), k=1)
    for b in range(B):
        q = (x[b] @ Wq.T + bq).reshape(S, H, HD).transpose(1, 0, 2)
        k = (x[b] @ Wk.T + bk).reshape(S, H, HD).transpose(1, 0, 2)
        v = (x[b] @ Wv.T + bv).reshape(S, H, HD).transpose(1, 0, 2)
        o = np.empty((H, S, HD), np.float32)
        pad = (attention_mask[b] == 0)[None, :]
        for h in range(H):
            s = (q[h] @ k[h].T) * scale
            s[causal] = -np.inf
            s = np.where(pad, np.float32(-1e9), s)
            s -= s.max(-1, keepdims=True)
            e = np.exp(s)
            p = e / e.sum(-1, keepdims=True)
            o[h] = p @ v[h]
        ctx = o.transpose(1, 0, 2).reshape(S, D)
        out[b] = ctx @ Wo.T + bo
    return out


def kernel(x, Wq, bq, Wk, bk, Wv, bv, Wo, bo, attention_mask):
    x = np.asarray(x, np.float32)
    Wq, bq = np.asarray(Wq, np.float32), np.asarray(bq, np.float32)
    Wk, bk = np.asarray(Wk, np.float32), np.asarray(bk, np.float32)
    Wv, bv = np.asarray(Wv, np.float32), np.asarray(bv, np.float32)
    Wo, bo = np.asarray(Wo, np.float32), np.asarray(bo, np.float32)
    attention_mask = np.asarray(attention_mask)

    if not np.all(attention_mask == 1):
        return _reference_numpy(x, Wq, bq, Wk, bk, Wv, bv, Wo, bo, attention_mask)

    nc = _build()

    xts = [np.ascontiguousarray(x[b].T).astype(ml_dtypes.bfloat16) for b in range(B)]
    xt8s = [np.ascontiguousarray((QK_SCALE * x[b].T)).astype(NP_FP8) for b in range(B)]
    shards = []
    for g in range(2):
        cs = slice(g * C, (g + 1) * C)
        shards.append(
            dict(
                wq8=np.ascontiguousarray((QK_SCALE * Wq[cs, :][_QK_PERM]).T).astype(NP_FP8),
                wk8=np.ascontiguousarray((QK_SCALE * Wk[cs, :][_QK_PERM]).T).astype(NP_FP8),
                wvt=np.ascontiguousarray(Wv[cs, :].T).astype(ml_dtypes.bfloat16),
                bqv=np.ascontiguousarray(QK_SCALE * bq[cs][_QK_PERM]),
                bvb=np.ascontiguousarray(np.broadcast_to(bv[cs], (128, C))),
                wot=np.ascontiguousarray(Wo[:, cs].T).astype(ml_dtypes.bfloat16),
            )
        )
    in_maps = []
    for c in range(N_CORES):
        b, g = c // 2, c % 2
        in_maps.append(dict(xt=xts[b], x8t=xt8s[b], **shards[g]))

    res = run_bass_kernel_spmd(nc, in_maps, core_ids=list(range(N_CORES)))

    out = np.empty((B, S, D), np.float32)
    for b in range(B):
        out[b] = (
            np.asarray(res.results[2 * b]["out"], np.float32)
            + np.asarray(res.results[2 * b + 1]["out"], np.float32)
            + bo
        )
    return out

